# revision 3
# baseline (speedup 1.0000x reference)
"""Paged causal GQA attention on 8 TRN2 NeuronCores.

Problem: query [8192, 32, 128] f32 (8 seqs x 1024 tokens), paged KV cache
[32 blocks, 256, 8, 128] f32, block_tables [8, 4] int32, causal attention
with GQA (32 q-heads, 8 kv-heads, n_rep=4), scale = 1/sqrt(128).

Sharding: one sequence per core. The paged-cache gather (block_tables) is
done host-side while slicing per-core inputs, so each core runs a dense
causal attention over its own 1024-token sequence. No collectives.

Per-core kernel layout (bf16 on-chip, f32 PSUM/output):
  qT  [32, 128, 1024]  per-head Q^T (d on partitions)
  kT  [8, 128, 1024]   per-kv-head K^T (d on partitions)
  v   [8, 1024, 128]   per-kv-head V (k on partitions)
  out [1024, 32, 128]  f32

Per head h (kvh = h//4), k-tiles j = 0..7 (128 keys each):
  S^T[k, q] = sum_d K^T[d,k] * Q^T[d,q]  for q >= 128j (causal tiles only),
  computed in three 1536-col passes over two rotating 3-bank PSUM slots,
  with a column layout that puts every causal-diagonal 128-block at a
  512-column stride in the P^T buffer.
  P^T = exp(scale * S^T) on ScalarE (3 big ops/head, PSUM -> SBUF bf16) —
  ScalarE is the bottleneck engine (~148us busy), so everything else is
  scheduled around keeping it saturated.
  One strided tensor_mul per head zeroes the 8 diagonal mask triangles.
  out[q, d] = sum_j P^T_j.T @ V'_j accumulated in PSUM; V' has a ones
  column appended so PV column 128 is the softmax denominator for free.
  VectorE: reciprocal + per-partition scale evacuates PSUM -> SBUF f32.
  PV work is software-pipelined: head h-1's q-tile pairs are emitted
  between head h's score passes so the PE never starves ScalarE.
  Outputs are normalized into one bf16 [128, 1024] staging tile per head
  and leave in a single DMA per head (32 dma_starts instead of 128): the
  SP sequencer pays ~565ns of issue time per dma_start, so fewer, larger
  stores shorten both the steady-state issue load and the kernel tail.
  kernel() upcasts the bf16 result to f32 on the host.
"""

import os
import sys

for _p in ("/opt/trn_rl_repo", "/root/.axon_site/_ro/trn_rl_repo"):
    if os.path.isdir(_p) and _p not in sys.path:
        sys.path.insert(0, _p)

import numpy as np
import ml_dtypes

BF16 = ml_dtypes.bfloat16

NUM_HEADS = 32
HEAD_DIM = 128
NUM_KV_HEADS = 8
N_REP = NUM_HEADS // NUM_KV_HEADS
SCALE = 0.08838834764831845
NUM_SEQS = 8
SEQ_LEN = 1024
NT = SEQ_LEN // 128  # 8 k/q tiles per sequence
N_CORES = 8

# Score chunks: (j, qstart, width, pass, col). Each pass fills one
# [128, 1536] (3-bank) PSUM slot. The first chunk of every j starts at the
# causal diagonal (q = 128j) and sits at a 512-aligned column; with pass
# bases 0/1536/3072 in P^T, diagonal blocks land at P^T columns 512*j for
# all j (one strided mask op). Tail chunks fill leftover bank space exactly.
CHUNKS = [
    (0, 0, 512, 0, 0),
    (1, 128, 512, 0, 512),
    (2, 256, 512, 0, 1024),
    (3, 384, 512, 1, 0),
    (4, 512, 512, 1, 512),
    (5, 640, 384, 1, 1024),
    (3, 896, 128, 1, 1408),
    (6, 768, 256, 2, 0),
    (2, 768, 256, 2, 256),
    (7, 896, 128, 2, 512),
    (1, 640, 384, 2, 640),
    (0, 512, 512, 2, 1024),
]
PASS_W = [1536, 1536, 1536]
PASS_BASE = [0, 1536, 3072]  # P^T column base per pass
PH_W = 4608


def _pcol(j, i):
    """Column in the P^T head buffer holding q-tile i of k-tile j."""
    for jj, qs, w, pa, col in CHUNKS:
        if jj == j and qs <= 128 * i < qs + w:
            return PASS_BASE[pa] + col + (128 * i - qs)
    raise AssertionError((j, i))


def _build_nc():
    import concourse.bacc as bacc
    import concourse.tile as tile
    import concourse.mybir as mybir

    f32 = mybir.dt.float32
    bf16 = mybir.dt.bfloat16
    Exp = mybir.ActivationFunctionType.Exp

    nc = bacc.Bacc("TRN2", target_bir_lowering=False, debug=False,
                   num_devices=N_CORES)

    qT = nc.dram_tensor("qT", [NUM_HEADS, HEAD_DIM, SEQ_LEN], bf16,
                        kind="ExternalInput").ap()
    kT = nc.dram_tensor("kT", [NUM_KV_HEADS, HEAD_DIM, SEQ_LEN], bf16,
                        kind="ExternalInput").ap()
    v = nc.dram_tensor("v", [NUM_KV_HEADS, SEQ_LEN, HEAD_DIM], bf16,
                       kind="ExternalInput").ap()
    mask = nc.dram_tensor("mask", [128, 128], bf16,
                          kind="ExternalInput").ap()
    out = nc.dram_tensor("out", [SEQ_LEN, NUM_HEADS, HEAD_DIM], bf16,
                         kind="ExternalOutput").ap()

    # first-in-bank chunks clear the bank's has_written bits (start=True)
    start_flag = {}
    seen_banks = set()
    for jj, qs, w, pa, col in CHUNKS:
        bank = (pa, col // 512)
        start_flag[(jj, qs)] = bank not in seen_banks
        seen_banks.add(bank)

    with tile.TileContext(nc) as tc:
        with (
            tc.tile_pool(name="qpool", bufs=NUM_HEADS) as qpool,
            tc.tile_pool(name="kpool", bufs=NUM_KV_HEADS) as kpool,
            tc.tile_pool(name="vpool", bufs=NUM_KV_HEADS) as vpool,
            tc.tile_pool(name="cpool", bufs=1) as cpool,
            tc.tile_pool(name="ppool", bufs=2) as ppool,
            tc.tile_pool(name="opool", bufs=3) as opool,
            tc.tile_pool(name="rpool", bufs=6) as rpool,
            tc.tile_pool(name="scpool", bufs=2, space="PSUM") as scpool,
            tc.tile_pool(name="pvpool", bufs=2, space="PSUM") as pvpool,
        ):
            mk = cpool.tile([128, 128], bf16, tag="mk")

            wu = cpool.tile([128, 512], bf16, tag="wu")
            nc.vector.memset(wu[:, :], 0.0)
            sc_wu = scpool.tile([128, 1536], f32, tag="sc")
            for _ in range(8):
                nc.tensor.matmul(sc_wu[:, 0:512], lhsT=wu[:, 0:128],
                                 rhs=wu[:, 0:512], start=True, stop=True,
                                 skip_group_check=True)

            kts = [None] * NUM_KV_HEADS
            vts = [None] * NUM_KV_HEADS
            qts = [None] * NUM_HEADS

            def load_k(kvh):
                kt_t = kpool.tile([128, SEQ_LEN], bf16, tag="kt")
                nc.sync.dma_start(out=kt_t[:, :], in_=kT[kvh])
                kts[kvh] = kt_t

            def load_v(kvh):
                # V' tile: 8 blocks of 129 cols (128 V cols + ones col)
                vt = vpool.tile([128, NT * 129], bf16, tag="vt")
                vt3 = vt[:, :].rearrange("p (j c) -> p j c", c=129)
                src = v[kvh].rearrange("(j p) d -> p j d", p=128)
                nc.sync.dma_start(out=vt3[:, :, 0:128], in_=src)
                nc.vector.memset(vt3[:, :, 128:129], 1.0)
                vts[kvh] = vt

            def emit_scores(h, last=False):
                """QK passes + exp + diagonal mask for head h; returns P^T."""
                kvh = h // N_REP
                qt = qpool.tile([128, SEQ_LEN], bf16, tag="qt")
                if h == 0:
                    # interleave the first K/Q loads in pass-0 consumption
                    # order so the first matmuls start as early as possible
                    kt_t = kpool.tile([128, SEQ_LEN], bf16, tag="kt")
                    kts[0] = kt_t
                    nc.sync.dma_start(out=kt_t[:, 0:384], in_=kT[0][:, 0:384])
                    nc.sync.dma_start(out=qt[:, 0:512], in_=qT[0][:, 0:512])
                    nc.sync.dma_start(out=qt[:, 512:1024],
                                      in_=qT[0][:, 512:1024])
                    nc.sync.dma_start(out=kt_t[:, 384:1024],
                                      in_=kT[0][:, 384:1024])
                    nc.sync.dma_start(out=mk[:, :], in_=mask[:, :])
                    load_v(0)
                else:
                    if kts[kvh] is None:
                        load_k(kvh)
                    if h < 4:
                        # early heads: halve transfers so Q arrives sooner
                        nc.sync.dma_start(out=qt[:, 0:512],
                                          in_=qT[h][:, 0:512])
                        nc.sync.dma_start(out=qt[:, 512:1024],
                                          in_=qT[h][:, 512:1024])
                    else:
                        nc.sync.dma_start(out=qt[:, :], in_=qT[h])
                if h % N_REP == 1 and kvh + 1 < NUM_KV_HEADS:
                    # prefetch the next kv head's K/V well before first use
                    load_k(kvh + 1)
                    load_v(kvh + 1)
                qts[h] = qt
                kt_t = kts[kvh]

                ph = ppool.tile([128, PH_W], bf16, tag="ph")
                return qt, kt_t, ph

            def emit_pass(qt, kt_t, ph, pa, split_act=False):
                sc = scpool.tile([128, 1536], f32, tag="sc")
                for jj, qs, w, cpa, col in CHUNKS:
                    if cpa != pa:
                        continue
                    nc.tensor.matmul(
                        sc[:, col:col + w],
                        lhsT=kt_t[:, 128 * jj:128 * jj + 128],
                        rhs=qt[:, qs:qs + w],
                        start=start_flag[(jj, qs)], stop=True,
                        skip_group_check=True,
                    )
                    if split_act:
                        # head 0 warm-up: exp each 512-col bank as soon as
                        # its matmul lands so ScalarE starts ~2.5us earlier
                        nc.scalar.activation(
                            ph[:, PASS_BASE[pa] + col:PASS_BASE[pa] + col + w],
                            sc[:, col:col + w], Exp, scale=SCALE)
                if split_act:
                    return
                pw = PASS_W[pa]
                pb = PASS_BASE[pa]
                nc.scalar.activation(
                    ph[:, pb:pb + pw], sc[:, 0:pw], Exp, scale=SCALE)

            def emit_mask(ph, j0, nd):
                # zero diagonal mask triangles j0..j0+nd (at 512-col stride)
                phd = ph[:, :].rearrange("p (j c) -> p j c", c=512)
                nc.vector.tensor_mul(
                    phd[:, j0:j0 + nd, 0:128], phd[:, j0:j0 + nd, 0:128],
                    mk[:, :].rearrange("p (o c) -> p o c", o=1).broadcast_to(
                        [128, nd, 128]))

            def emit_pv_chain(h, ph, pv, i, base, first_start):
                vt = vts[h // N_REP]
                for j in range(i + 1):
                    c = _pcol(j, i)
                    nc.tensor.matmul(
                        pv[:, base:base + 129],
                        lhsT=ph[:, c:c + 128],
                        rhs=vt[:, 129 * j:129 * j + 129],
                        start=(j == 0 and first_start),
                        stop=(j == i),
                        skip_group_check=True,
                    )

            osbs = {}
            osb_done = {}

            def emit_pv_pair(h, ph, p, only=None):
                """PV + normalize + store for q-tile pair p of head h. The
                pair shares one PSUM bank (even chain at cols 0:129, odd at
                129:258); column 128/257 is the softmax denominator.
                only='even' emits just the even chain and returns the psum
                tile; pass it back via only=(pv,) to finish the pair."""
                ie, io = 2 * p, 2 * p + 1
                if only == "even":
                    pv = pvpool.tile([128, 258], f32, tag="pv")
                    emit_pv_chain(h, ph, pv, ie, 0, True)
                    return pv
                if isinstance(only, tuple):
                    pv = only[0]
                    emit_pv_chain(h, ph, pv, io, 129, False)
                else:
                    pv = pvpool.tile([128, 258], f32, tag="pv")
                    emit_pv_chain(h, ph, pv, ie, 0, True)
                    emit_pv_chain(h, ph, pv, io, 129, False)
                r = rpool.tile([128, 2], f32, tag="r")
                pv3 = pv[:, :].rearrange("p (t c) -> p t c", c=129)
                nc.vector.reciprocal(r[:, :], pv3[:, :, 128])
                # normalize into a shared per-head staging tile; one DMA per
                # head (instead of 4) keeps the SP sequencer's ~565ns-per-
                # dma_start issue cost off the critical path
                if h not in osbs:
                    osb_t = opool.tile([128, 1024], bf16, tag="osb")
                    osbs[h] = osb_t
                    osb_done[h] = 0
                osb = osbs[h]
                # one tensor_mul per pair (not two tensor_scalar_muls): r is
                # broadcast along d with a stride-0 AP, halving the DVE
                # instruction count of the normalize path
                osb3 = osb[:, 256 * p:256 * p + 256].rearrange(
                    "p (t d) -> p t d", d=128)
                nc.vector.tensor_mul(
                    osb3, pv3[:, :, 0:128],
                    r[:, :].rearrange("p (t o) -> p t o", o=1).broadcast_to(
                        [128, 2, 128]))
                osb_done[h] += 1
                if osb_done[h] == 4:
                    dst = out[:, h, :].rearrange("(t q) d -> q t d", t=8)
                    nc.sync.dma_start(
                        out=dst,
                        in_=osb[:, :].rearrange("p (t d) -> p t d", d=128))

            # Software pipeline at q-tile-pair granularity: head h-1's PV
            # pairs are emitted between head h's score passes so the PE
            # stream never starves ScalarE for a whole PV block. The last
            # head uses per-pass mask unlock to shorten the kernel tail.
            prev = None
            for h in range(NUM_HEADS):
                last = h == NUM_HEADS - 1
                qt, kt_t, ph = emit_scores(h, last)
                for pa in range(3):
                    emit_pass(qt, kt_t, ph, pa)
                    if last:
                        emit_mask(ph, 3 * pa, 3 if pa < 2 else 2)
                    if prev is not None:
                        emit_pv_pair(prev[0], prev[1], pa + 1)
                    if last and pa >= 1:
                        # pair p needs passes <= p: pair0 after pass1's
                        # mask, pair1 after pass2's (tiles 2,3 use pass1)
                        emit_pv_pair(h, ph, pa - 1)
                if not last:
                    emit_mask(ph, 0, 8)
                if prev is not None:
                    emit_pv_pair(prev[0], prev[1], 0)
                prev = (h, ph)
            for p in range(2, 4):
                emit_pv_pair(prev[0], prev[1], p)

    nc.compile()
    return nc


_NC_CACHE = {}


def _get_nc():
    if "nc" not in _NC_CACHE:
        _NC_CACHE["nc"] = _build_nc()
    return _NC_CACHE["nc"]


def make_in_maps(query, k_cache, v_cache, block_tables):
    query = np.asarray(query, dtype=np.float32)
    k_cache = np.asarray(k_cache, dtype=np.float32)
    v_cache = np.asarray(v_cache, dtype=np.float32)
    block_tables = np.asarray(block_tables)

    # mask[k, q] = 1 where q >= k (keep), 0 where q < k (causal-masked)
    mask = (np.arange(128)[None, :] >= np.arange(128)[:, None]).astype(BF16)

    in_maps = []
    for i in range(N_CORES):
        q_i = query[SEQ_LEN * i:SEQ_LEN * (i + 1)]  # [1024, 32, 128]
        qT_i = np.ascontiguousarray(
            q_i.transpose(1, 2, 0)).astype(BF16)  # [32, 128, 1024]
        blocks = block_tables[i]
        k_i = k_cache[blocks].reshape(SEQ_LEN, NUM_KV_HEADS, HEAD_DIM)
        v_i = v_cache[blocks].reshape(SEQ_LEN, NUM_KV_HEADS, HEAD_DIM)
        kT_i = np.ascontiguousarray(k_i.transpose(1, 2, 0)).astype(BF16)
        vv_i = np.ascontiguousarray(v_i.transpose(1, 0, 2)).astype(BF16)
        in_maps.append({
            "qT": qT_i, "kT": kT_i, "v": vv_i, "mask": mask,
        })
    return in_maps


def kernel(query, k_cache, v_cache, block_tables):
    from concourse.bass_utils import run_bass_kernel_spmd

    in_maps = make_in_maps(query, k_cache, v_cache, block_tables)
    nc = _get_nc()
    res = run_bass_kernel_spmd(nc, in_maps, list(range(N_CORES)))
    outs = [np.asarray(res.results[i]["out"]) for i in range(N_CORES)]
    return np.concatenate(outs, axis=0).astype(np.float32)

